# revision 7
# baseline (speedup 1.0000x reference)
"""Contrastive-loss kernel for 8 Trainium2 NeuronCores (SPMD, Bass/Tile).

Strategy (data-parallel over rows of the 4096x4096 similarity matrix):
  - Each core owns 512 rows (4 stripes of 128). It receives the full feature
    matrix, column-PERMUTED per core so its positive-pair blocks sit at
    program-constant offsets: perm = [own-view 512-block, other-view
    512-block, rest]. All core-dependence lives in input data (SPMD-safe).
  - featsT is pre-scaled by sqrt(TEMP) so PE matmuls produce logits directly.
  - Per stripe: two [128,2048] PSUM groups of matmuls (float32r, full rate);
    rank-1 bf16-free fixup matmuls subtract BIG=25 on same-class blocks so
    exp/max row-reductions see negatives only.
  - ACT: exp with fused row-accumulate -> neg_sum. DVE: strided
    tensor_tensor_reduce max chain -> max_neg (as thr = max_neg - BIG).
  - Positive blocks are read straight from PSUM (post-fixup, values z-BIG):
    fused compare+count (correct) and weighted-sum (pos logits) ops.
  - Host: tiny label math, final log/sum/divide on 4096-length vectors.
"""
import sys

if "/opt/trn_rl_repo" not in sys.path:
    sys.path.insert(0, "/opt/trn_rl_repo")

from contextlib import ExitStack

import numpy as np

import concourse.bass as bass
import concourse.tile as tile
from concourse import bacc, mybir
from concourse.bass_utils import run_bass_kernel_spmd

F32 = mybir.dt.float32
AX = mybir.AxisListType
OP = mybir.AluOpType
ACTF = mybir.ActivationFunctionType

K = 32
TEMP = 0.01
OTHER = 0.5
BS = 64
F = 128
N1 = 2048
N = 4096
NC = 8
RPC = 512          # rows per core
NSTRIPE = 4
BIG = 25.0
SQB = 5.0          # sqrt(BIG)

_CACHE: dict = {}


def _build_nc():
    nc = bacc.Bacc("TRN2", target_bir_lowering=False, debug=False, num_devices=NC)

    F32R = mybir.dt.float32r
    fT_d = nc.dram_tensor("featsT", [F, N], F32R, kind="ExternalInput").ap()
    vm_d = nc.dram_tensor("vmask", [128, 1024], F32, kind="ExternalInput").ap()
    wm_d = nc.dram_tensor("wmask", [128, 1024], F32, kind="ExternalInput").ap()
    ovf_d = nc.dram_tensor("ovfix", [1, 512], F32, kind="ExternalInput").ap()

    negsum_d = nc.dram_tensor("negsum", [128, 4], F32, kind="ExternalOutput").ap()
    thr_d = nc.dram_tensor("thr", [128, 4], F32, kind="ExternalOutput").ap()
    possum_d = nc.dram_tensor("possum", [128, 4], F32, kind="ExternalOutput").ap()
    corr_d = nc.dram_tensor("corr", [128, 4], F32, kind="ExternalOutput").ap()

    with tile.TileContext(nc) as tc, ExitStack() as ctx:
        singles = ctx.enter_context(tc.tile_pool(name="singles", bufs=1))
        psum = ctx.enter_context(tc.tile_pool(name="psum", bufs=2, space="PSUM"))
        expp = ctx.enter_context(tc.tile_pool(name="expp", bufs=2))
        maxp = ctx.enter_context(tc.tile_pool(name="maxp", bufs=2))
        posp = ctx.enter_context(tc.tile_pool(name="posp", bufs=2))
        stat = ctx.enter_context(tc.tile_pool(name="stat", bufs=3))
        outp = ctx.enter_context(tc.tile_pool(name="outs", bufs=1))

        fT_sb = singles.tile([F, N], F32R)
        for t in range(8):
            nc.sync.dma_start(fT_sb[:, 512 * t:512 * (t + 1)],
                              fT_d[:, 512 * t:512 * (t + 1)])
        vm_sb = singles.tile([128, 1024], F32)
        nc.sync.dma_start(vm_sb[:], vm_d[:])
        wm_sb = singles.tile([128, 1024], F32)
        nc.sync.dma_start(wm_sb[:], wm_d[:])
        ovf_sb = singles.tile([1, 512], F32)
        nc.sync.dma_start(ovf_sb[:], ovf_d[:])
        ones_pos = singles.tile([1, 64], F32)
        nc.vector.memset(ones_pos[:], SQB)
        ones_neg = singles.tile([1, 64], F32)
        nc.vector.memset(ones_neg[:], -SQB)

        negsum_sb = outp.tile([128, 4], F32)
        thr_sb = outp.tile([128, 4], F32)
        possum_sb = outp.tile([128, 4], F32)
        corr_sb = outp.tile([128, 4], F32)

        fTr = fT_sb[:]

        for s in range(NSTRIPE):
            lhsT = fTr[:, 128 * s:128 * s + 128]
            zg = [psum.tile([128, 2048], F32, tag="zg", name=f"zg{s}_{g}")
                  for g in range(2)]
            for g in range(2):
                for t4 in range(4):
                    nc.tensor.matmul(
                        zg[g][:, 512 * t4:512 * (t4 + 1)],
                        lhsT,
                        fTr[:, 2048 * g + 512 * t4:2048 * g + 512 * (t4 + 1)],
                        start=True, stop=True)
            # fixups: subtract BIG on same-class blocks (group 0 only).
            for h in range(2):
                u = 2 * s + h
                nc.tensor.matmul(
                    zg[0][64 * h:64 * h + 64, 64 * u:64 * u + 64],
                    ones_pos[:], ones_neg[:],
                    start=False, stop=True, skip_group_check=True)
                nc.tensor.matmul(
                    zg[0][64 * h:64 * h + 64, 512 + 64 * u:512 + 64 * u + 64],
                    ones_pos[:], ovf_sb[:, 64 * u:64 * u + 64],
                    start=False, stop=True, skip_group_check=True)

            negparts = stat.tile([128, 2], F32)
            maxch = stat.tile([128, 2], F32)
            for g in range(2):
                ex = expp.tile([128, 2048], F32, tag="ex", name=f"ex{s}_{g}")
                nc.scalar.activation(ex[:], zg[g][:], ACTF.Exp,
                                     accum_out=negparts[:, g:g + 1])
                nc.vector.reduce_max(maxch[:, g:g + 1], zg[g][:], axis=AX.X)
            nc.vector.reduce_sum(negsum_sb[:, s:s + 1], negparts[:], axis=AX.X)
            # thr = max_neg - BIG (pos entries in zg hold z - BIG after fixup)
            mxall = stat.tile([128, 1], F32)
            nc.vector.reduce_max(mxall[:], maxch[:], axis=AX.X)
            nc.vector.tensor_scalar_add(thr_sb[:, s:s + 1], mxall[:], -BIG)

            corrparts = stat.tile([128, 2], F32)
            posparts = stat.tile([128, 2], F32)
            for b in range(2):
                region = zg[0][:, 512 * b + 128 * s:512 * b + 128 * s + 128]
                msl = slice(256 * s + 128 * b, 256 * s + 128 * b + 128)
                sc1 = posp.tile([128, 128], F32, tag="sc1")
                nc.vector.scalar_tensor_tensor(
                    out=sc1[:], in0=region, scalar=thr_sb[:, s:s + 1],
                    in1=vm_sb[:, msl], op0=OP.is_gt, op1=OP.mult,
                    accum_out=corrparts[:, b:b + 1])
                sc2 = posp.tile([128, 128], F32, tag="sc2")
                nc.vector.scalar_tensor_tensor(
                    out=sc2[:], in0=region, scalar=1.0,
                    in1=wm_sb[:, msl], op0=OP.mult, op1=OP.mult,
                    accum_out=posparts[:, b:b + 1])
            nc.vector.reduce_sum(corr_sb[:, s:s + 1], corrparts[:], axis=AX.X)
            nc.vector.reduce_sum(possum_sb[:, s:s + 1], posparts[:], axis=AX.X)

        nc.sync.dma_start(negsum_d[:], negsum_sb[:])
        nc.sync.dma_start(thr_d[:], thr_sb[:])
        nc.sync.dma_start(possum_d[:], possum_sb[:])
        nc.sync.dma_start(corr_d[:], corr_sb[:])

    nc.compile()
    return nc


def _host_prep(feats1, feats2, overlap_inds):
    feats = np.concatenate([np.asarray(feats1, np.float32),
                            np.asarray(feats2, np.float32)], 0)
    featsT = np.ascontiguousarray(feats.T * np.float32(np.sqrt(TEMP)))
    ov = np.asarray(overlap_inds, bool)
    eye128 = np.eye(128, dtype=np.float32)

    in_maps = []
    wcnts, vcnts = [], []
    for c in range(NC):
        view2 = c >= 4
        cc = c - 4 if view2 else c
        self_s = 2048 + 512 * cc if view2 else 512 * cc
        other_s = 512 * cc if view2 else 2048 + 512 * cc
        keep = np.ones(N, bool)
        keep[self_s:self_s + 512] = False
        keep[other_s:other_s + 512] = False
        perm = np.concatenate([np.arange(self_s, self_s + 512),
                               np.arange(other_s, other_s + 512),
                               np.nonzero(keep)[0]])
        fT_c = np.ascontiguousarray(featsT[:, perm])

        V = np.zeros((128, NSTRIPE, 2, 128), np.float32)
        W = np.zeros((128, NSTRIPE, 2, 128), np.float32)
        ovfix = np.zeros((1, 512), np.float32)
        for s in range(NSTRIPE):
            for h in range(2):
                u = 2 * s + h
                m = 8 * cc + u
                rows = slice(64 * h, 64 * h + 64)
                lo = 64 * u - 128 * s
                V[rows, s, 0, lo:lo + 64] = 1.0
                W[rows, s, 0, lo:lo + 64] = 1.0
                if ov[m]:
                    V[rows, s, 1, lo:lo + 64] = 1.0
                    W[rows, s, 1, lo:lo + 64] = OTHER
                    ovfix[0, 64 * u:64 * u + 64] = -SQB
            V[:, s, 0, :] *= (1 - eye128)
            W[:, s, 0, :] *= (1 - eye128)

        wcnts.append(W.reshape(128, NSTRIPE, 256).sum(-1))
        vcnts.append(V.reshape(128, NSTRIPE, 256).sum(-1))
        in_maps.append({
            "featsT": fT_c,
            "vmask": np.ascontiguousarray(V.reshape(128, 1024)),
            "wmask": np.ascontiguousarray(W.reshape(128, 1024)),
            "ovfix": ovfix,
        })
    return in_maps, wcnts, vcnts


def kernel(feats1, feats2, overlap_inds, bs):
    assert int(bs) == BS
    feats1 = np.asarray(feats1, np.float32)
    feats2 = np.asarray(feats2, np.float32)
    assert feats1.shape == (N1, F) and feats2.shape == (N1, F)

    in_maps, wcnts, vcnts = _host_prep(feats1, feats2, overlap_inds)

    if "nc" not in _CACHE:
        _CACHE["nc"] = _build_nc()
    res = run_bass_kernel_spmd(_CACHE["nc"], in_maps, list(range(NC)))

    total_loss = 0.0
    total_corr = 0.0
    total_pos = 0.0
    for c in range(NC):
        out = res.results[c]
        negsum = out["negsum"].astype(np.float64)
        possum = out["possum"].astype(np.float64) + BIG * wcnts[c]
        corr = out["corr"].astype(np.float64)
        wcnt = wcnts[c].astype(np.float64)
        total_loss += (wcnt * np.log(negsum) - possum).sum()
        total_corr += corr.sum()
        total_pos += vcnts[c].sum(dtype=np.float64)

    loss = np.float32(total_loss / total_pos)
    acc = np.float32(total_corr / total_pos)
    return acc, loss


# revision 9
# speedup vs baseline: 1.0679x; 1.0679x over previous
"""Contrastive-loss kernel for 8 Trainium2 NeuronCores (SPMD, Bass/Tile).

Strategy (data-parallel over rows of the 4096x4096 similarity matrix):
  - Each core owns 512 rows (4 stripes of 128). It receives the full feature
    matrix, column-PERMUTED per core so its positive-pair blocks sit at
    program-constant offsets: perm = [own-view 512-block, other-view
    512-block, rest]. All core-dependence lives in input data (SPMD-safe).
  - featsT is pre-scaled by sqrt(TEMP) so PE matmuls produce logits directly.
  - Per stripe: two [128,2048] PSUM groups of matmuls (float32r, full rate);
    rank-1 bf16-free fixup matmuls subtract BIG=25 on same-class blocks so
    exp/max row-reductions see negatives only.
  - ACT: exp with fused row-accumulate -> neg_sum. DVE: strided
    tensor_tensor_reduce max chain -> max_neg (as thr = max_neg - BIG).
  - Positive blocks are read straight from PSUM (post-fixup, values z-BIG):
    fused compare+count (correct) and weighted-sum (pos logits) ops.
  - Host: tiny label math, final log/sum/divide on 4096-length vectors.
"""
import sys

if "/opt/trn_rl_repo" not in sys.path:
    sys.path.insert(0, "/opt/trn_rl_repo")

from contextlib import ExitStack

import numpy as np

import concourse.bass as bass
import concourse.tile as tile
from concourse import bacc, mybir
from concourse.bass_utils import run_bass_kernel_spmd

F32 = mybir.dt.float32
AX = mybir.AxisListType
OP = mybir.AluOpType
ACTF = mybir.ActivationFunctionType

K = 32
TEMP = 0.01
OTHER = 0.5
BS = 64
F = 128
N1 = 2048
N = 4096
NC = 8
RPC = 512          # rows per core
NSTRIPE = 4
BIG = 25.0
SQB = 5.0          # sqrt(BIG)

_CACHE: dict = {}


def _build_nc():
    nc = bacc.Bacc("TRN2", target_bir_lowering=False, debug=False, num_devices=NC)

    F32R = mybir.dt.float32r
    fT_d = nc.dram_tensor("featsT", [F, N], F32R, kind="ExternalInput").ap()
    vm_d = nc.dram_tensor("vmask", [128, 1024], F32, kind="ExternalInput").ap()
    wm_d = nc.dram_tensor("wmask", [128, 1024], F32, kind="ExternalInput").ap()
    ovf_d = nc.dram_tensor("ovfix", [1, 512], F32, kind="ExternalInput").ap()

    negsum_d = nc.dram_tensor("negsum", [128, 4], F32, kind="ExternalOutput").ap()
    thr_d = nc.dram_tensor("thr", [128, 4], F32, kind="ExternalOutput").ap()
    possum_d = nc.dram_tensor("possum", [128, 4], F32, kind="ExternalOutput").ap()
    corr_d = nc.dram_tensor("corr", [128, 4], F32, kind="ExternalOutput").ap()

    with tile.TileContext(nc) as tc, ExitStack() as ctx:
        singles = ctx.enter_context(tc.tile_pool(name="singles", bufs=1))
        psum = ctx.enter_context(tc.tile_pool(name="psum", bufs=2, space="PSUM"))
        expp = ctx.enter_context(tc.tile_pool(name="expp", bufs=2))
        maxp = ctx.enter_context(tc.tile_pool(name="maxp", bufs=2))
        posp = ctx.enter_context(tc.tile_pool(name="posp", bufs=2))
        stat = ctx.enter_context(tc.tile_pool(name="stat", bufs=3))
        outp = ctx.enter_context(tc.tile_pool(name="outs", bufs=1))

        fT_sb = singles.tile([F, N], F32R)
        for t in range(8):
            nc.sync.dma_start(fT_sb[:, 512 * t:512 * (t + 1)],
                              fT_d[:, 512 * t:512 * (t + 1)])
        vm_sb = singles.tile([128, 1024], F32)
        nc.sync.dma_start(vm_sb[:], vm_d[:])
        wm_sb = singles.tile([128, 1024], F32)
        nc.sync.dma_start(wm_sb[:], wm_d[:])
        ovf_sb = singles.tile([1, 512], F32)
        nc.sync.dma_start(ovf_sb[:], ovf_d[:])
        ones_pos = singles.tile([1, 64], F32)
        nc.vector.memset(ones_pos[:], SQB)
        ones_neg = singles.tile([1, 64], F32)
        nc.vector.memset(ones_neg[:], -SQB)

        negsum_sb = outp.tile([128, 4], F32)
        thr_sb = outp.tile([128, 4], F32)
        possum_sb = outp.tile([128, 4], F32)
        corr_sb = outp.tile([128, 4], F32)

        fTr = fT_sb[:]

        for s in range(NSTRIPE):
            lhsT = fTr[:, 128 * s:128 * s + 128]
            zg = [psum.tile([128, 2048], F32, tag="zg", name=f"zg{s}_{g}")
                  for g in range(2)]
            # all 8 big matmuls back-to-back with the same stationary lhsT
            for g in range(2):
                for t4 in range(4):
                    nc.tensor.matmul(
                        zg[g][:, 512 * t4:512 * (t4 + 1)],
                        lhsT,
                        fTr[:, 2048 * g + 512 * t4:2048 * g + 512 * (t4 + 1)],
                        start=True, stop=True)
            # fixups: subtract BIG on same-class blocks (group 0 only);
            # emitted after all big matmuls so PE switches weights only once.
            for h in range(2):
                u = 2 * s + h
                nc.tensor.matmul(
                    zg[0][64 * h:64 * h + 64, 64 * u:64 * u + 64],
                    ones_pos[:], ones_neg[:],
                    start=False, stop=True, skip_group_check=True)
                nc.tensor.matmul(
                    zg[0][64 * h:64 * h + 64, 512 + 64 * u:512 + 64 * u + 64],
                    ones_pos[:], ovf_sb[:, 64 * u:64 * u + 64],
                    start=False, stop=True, skip_group_check=True)

            # copy the two positive-block regions out of PSUM early (ACT) so
            # the PSUM slot is released after exp+max, not after the pos ops
            posgath = posp.tile([128, 256], F32, tag="posgath")
            for b in range(2):
                nc.scalar.copy(
                    posgath[:, 128 * b:128 * b + 128],
                    zg[0][:, 512 * b + 128 * s:512 * b + 128 * s + 128])

            negparts = stat.tile([128, 2], F32)
            maxch = stat.tile([128, 2], F32)
            for g in range(2):
                ex = expp.tile([128, 2048], F32, tag="ex", name=f"ex{s}_{g}")
                nc.scalar.activation(ex[:], zg[g][:], ACTF.Exp,
                                     accum_out=negparts[:, g:g + 1])
                nc.vector.reduce_max(maxch[:, g:g + 1], zg[g][:], axis=AX.X)
            nc.vector.reduce_sum(negsum_sb[:, s:s + 1], negparts[:], axis=AX.X)
            mxall = stat.tile([128, 1], F32)
            nc.vector.reduce_max(mxall[:], maxch[:], axis=AX.X)
            # thr = max_neg - BIG (pos entries hold z - BIG after fixup)
            nc.vector.tensor_scalar_add(thr_sb[:, s:s + 1], mxall[:], -BIG)

            msl = slice(256 * s, 256 * s + 256)
            sc1 = posp.tile([128, 256], F32, tag="sc1")
            nc.vector.scalar_tensor_tensor(
                out=sc1[:], in0=posgath[:], scalar=thr_sb[:, s:s + 1],
                in1=vm_sb[:, msl], op0=OP.is_gt, op1=OP.mult,
                accum_out=corr_sb[:, s:s + 1])
            sc2 = posp.tile([128, 256], F32, tag="sc2")
            nc.vector.scalar_tensor_tensor(
                out=sc2[:], in0=posgath[:], scalar=1.0,
                in1=wm_sb[:, msl], op0=OP.mult, op1=OP.mult,
                accum_out=possum_sb[:, s:s + 1])

        nc.sync.dma_start(negsum_d[:], negsum_sb[:])
        nc.sync.dma_start(thr_d[:], thr_sb[:])
        nc.sync.dma_start(possum_d[:], possum_sb[:])
        nc.sync.dma_start(corr_d[:], corr_sb[:])

    nc.compile()
    return nc


def _host_prep(feats1, feats2, overlap_inds):
    feats = np.concatenate([np.asarray(feats1, np.float32),
                            np.asarray(feats2, np.float32)], 0)
    featsT = np.ascontiguousarray(feats.T * np.float32(np.sqrt(TEMP)))
    ov = np.asarray(overlap_inds, bool)
    eye128 = np.eye(128, dtype=np.float32)

    in_maps = []
    wcnts, vcnts = [], []
    for c in range(NC):
        view2 = c >= 4
        cc = c - 4 if view2 else c
        self_s = 2048 + 512 * cc if view2 else 512 * cc
        other_s = 512 * cc if view2 else 2048 + 512 * cc
        keep = np.ones(N, bool)
        keep[self_s:self_s + 512] = False
        keep[other_s:other_s + 512] = False
        perm = np.concatenate([np.arange(self_s, self_s + 512),
                               np.arange(other_s, other_s + 512),
                               np.nonzero(keep)[0]])
        fT_c = np.ascontiguousarray(featsT[:, perm])

        V = np.zeros((128, NSTRIPE, 2, 128), np.float32)
        W = np.zeros((128, NSTRIPE, 2, 128), np.float32)
        ovfix = np.zeros((1, 512), np.float32)
        for s in range(NSTRIPE):
            for h in range(2):
                u = 2 * s + h
                m = 8 * cc + u
                rows = slice(64 * h, 64 * h + 64)
                lo = 64 * u - 128 * s
                V[rows, s, 0, lo:lo + 64] = 1.0
                W[rows, s, 0, lo:lo + 64] = 1.0
                if ov[m]:
                    V[rows, s, 1, lo:lo + 64] = 1.0
                    W[rows, s, 1, lo:lo + 64] = OTHER
                    ovfix[0, 64 * u:64 * u + 64] = -SQB
            V[:, s, 0, :] *= (1 - eye128)
            W[:, s, 0, :] *= (1 - eye128)

        wcnts.append(W.reshape(128, NSTRIPE, 256).sum(-1))
        vcnts.append(V.reshape(128, NSTRIPE, 256).sum(-1))
        in_maps.append({
            "featsT": fT_c,
            "vmask": np.ascontiguousarray(V.reshape(128, 1024)),
            "wmask": np.ascontiguousarray(W.reshape(128, 1024)),
            "ovfix": ovfix,
        })
    return in_maps, wcnts, vcnts


def kernel(feats1, feats2, overlap_inds, bs):
    assert int(bs) == BS
    feats1 = np.asarray(feats1, np.float32)
    feats2 = np.asarray(feats2, np.float32)
    assert feats1.shape == (N1, F) and feats2.shape == (N1, F)

    in_maps, wcnts, vcnts = _host_prep(feats1, feats2, overlap_inds)

    if "nc" not in _CACHE:
        _CACHE["nc"] = _build_nc()
    res = run_bass_kernel_spmd(_CACHE["nc"], in_maps, list(range(NC)))

    total_loss = 0.0
    total_corr = 0.0
    total_pos = 0.0
    for c in range(NC):
        out = res.results[c]
        negsum = out["negsum"].astype(np.float64)
        possum = out["possum"].astype(np.float64) + BIG * wcnts[c]
        corr = out["corr"].astype(np.float64)
        wcnt = wcnts[c].astype(np.float64)
        total_loss += (wcnt * np.log(negsum) - possum).sum()
        total_corr += corr.sum()
        total_pos += vcnts[c].sum(dtype=np.float64)

    loss = np.float32(total_loss / total_pos)
    acc = np.float32(total_corr / total_pos)
    return acc, loss


# revision 10
# speedup vs baseline: 1.2760x; 1.1949x over previous
"""Contrastive-loss kernel for 8 Trainium2 NeuronCores (SPMD, Bass/Tile).

Strategy (data-parallel over rows of the 4096x4096 similarity matrix):
  - Each core owns 512 rows (4 stripes of 128). It receives the full feature
    matrix, column-PERMUTED per core so its positive-pair blocks sit at
    program-constant offsets: perm = [own-view 512-block, other-view
    512-block, rest]. All core-dependence lives in input data (SPMD-safe).
  - featsT is pre-scaled by sqrt(TEMP) so PE matmuls produce logits directly.
  - Per stripe: two [128,2048] PSUM groups of matmuls (float32r, full rate);
    rank-1 bf16-free fixup matmuls subtract BIG=25 on same-class blocks so
    exp/max row-reductions see negatives only.
  - ACT: exp with fused row-accumulate -> neg_sum. DVE: strided
    tensor_tensor_reduce max chain -> max_neg (as thr = max_neg - BIG).
  - Positive blocks are read straight from PSUM (post-fixup, values z-BIG):
    fused compare+count (correct) and weighted-sum (pos logits) ops.
  - Host: tiny label math, final log/sum/divide on 4096-length vectors.
"""
import sys

if "/opt/trn_rl_repo" not in sys.path:
    sys.path.insert(0, "/opt/trn_rl_repo")

from contextlib import ExitStack

import numpy as np

import concourse.bass as bass
import concourse.tile as tile
from concourse import bacc, mybir
from concourse.bass_utils import run_bass_kernel_spmd

F32 = mybir.dt.float32
AX = mybir.AxisListType
OP = mybir.AluOpType
ACTF = mybir.ActivationFunctionType

K = 32
TEMP = 0.01
OTHER = 0.5
BS = 64
F = 128
N1 = 2048
N = 4096
NC = 8
RPC = 512          # rows per core
NSTRIPE = 4
BIG = 25.0
SQB = 5.0          # sqrt(BIG)

_CACHE: dict = {}


def _build_nc():
    nc = bacc.Bacc("TRN2", target_bir_lowering=False, debug=False, num_devices=NC)

    F16 = mybir.dt.float16
    fT_d = nc.dram_tensor("featsT", [F, N], F16, kind="ExternalInput").ap()
    vm_d = nc.dram_tensor("vmask", [128, 1024], F32, kind="ExternalInput").ap()
    wm_d = nc.dram_tensor("wmask", [128, 1024], F32, kind="ExternalInput").ap()
    ovf_d = nc.dram_tensor("ovfix", [1, 512], F16, kind="ExternalInput").ap()

    negsum_d = nc.dram_tensor("negsum", [128, 4], F32, kind="ExternalOutput").ap()
    thr_d = nc.dram_tensor("thr", [128, 4], F32, kind="ExternalOutput").ap()
    possum_d = nc.dram_tensor("possum", [128, 4], F32, kind="ExternalOutput").ap()
    corr_d = nc.dram_tensor("corr", [128, 4], F32, kind="ExternalOutput").ap()

    with tile.TileContext(nc) as tc, ExitStack() as ctx:
        singles = ctx.enter_context(tc.tile_pool(name="singles", bufs=1))
        psum = ctx.enter_context(tc.tile_pool(name="psum", bufs=2, space="PSUM"))
        expp = ctx.enter_context(tc.tile_pool(name="expp", bufs=2))
        maxp = ctx.enter_context(tc.tile_pool(name="maxp", bufs=2))
        posp = ctx.enter_context(tc.tile_pool(name="posp", bufs=2))
        stat = ctx.enter_context(tc.tile_pool(name="stat", bufs=3))
        outp = ctx.enter_context(tc.tile_pool(name="outs", bufs=1))

        fT_sb = singles.tile([F, N], F16)
        for t in range(8):
            nc.sync.dma_start(fT_sb[:, 512 * t:512 * (t + 1)],
                              fT_d[:, 512 * t:512 * (t + 1)])
        vm_sb = singles.tile([128, 1024], F32)
        nc.sync.dma_start(vm_sb[:], vm_d[:])
        wm_sb = singles.tile([128, 1024], F32)
        nc.sync.dma_start(wm_sb[:], wm_d[:])
        ovf_sb = singles.tile([1, 512], F16)
        nc.sync.dma_start(ovf_sb[:], ovf_d[:])
        ones_pos = singles.tile([1, 64], F16)
        nc.vector.memset(ones_pos[:], SQB)
        ones_neg = singles.tile([1, 64], F16)
        nc.vector.memset(ones_neg[:], -SQB)

        negsum_sb = outp.tile([128, 4], F32)
        thr_sb = outp.tile([128, 4], F32)
        possum_sb = outp.tile([128, 4], F32)
        corr_sb = outp.tile([128, 4], F32)

        fTr = fT_sb[:]

        for s in range(NSTRIPE):
            lhsT = fTr[:, 128 * s:128 * s + 128]
            zg = [psum.tile([128, 2048], F32, tag="zg", name=f"zg{s}_{g}")
                  for g in range(2)]
            # all 8 big matmuls back-to-back with the same stationary lhsT
            for g in range(2):
                for t4 in range(4):
                    nc.tensor.matmul(
                        zg[g][:, 512 * t4:512 * (t4 + 1)],
                        lhsT,
                        fTr[:, 2048 * g + 512 * t4:2048 * g + 512 * (t4 + 1)],
                        start=True, stop=True)
            # fixups: subtract BIG on same-class blocks (group 0 only);
            # emitted after all big matmuls so PE switches weights only once.
            for h in range(2):
                u = 2 * s + h
                nc.tensor.matmul(
                    zg[0][64 * h:64 * h + 64, 64 * u:64 * u + 64],
                    ones_pos[:], ones_neg[:],
                    start=False, stop=True, skip_group_check=True)
                nc.tensor.matmul(
                    zg[0][64 * h:64 * h + 64, 512 + 64 * u:512 + 64 * u + 64],
                    ones_pos[:], ovf_sb[:, 64 * u:64 * u + 64],
                    start=False, stop=True, skip_group_check=True)

            # copy the two positive-block regions out of PSUM early (ACT) so
            # the PSUM slot is released after exp+max, not after the pos ops
            posgath = posp.tile([128, 256], F32, tag="posgath")
            for b in range(2):
                nc.scalar.copy(
                    posgath[:, 128 * b:128 * b + 128],
                    zg[0][:, 512 * b + 128 * s:512 * b + 128 * s + 128])

            negparts = stat.tile([128, 2], F32)
            maxch = stat.tile([128, 2], F32)
            for g in range(2):
                ex = expp.tile([128, 2048], F32, tag="ex", name=f"ex{s}_{g}")
                nc.scalar.activation(ex[:], zg[g][:], ACTF.Exp,
                                     accum_out=negparts[:, g:g + 1])
                nc.vector.reduce_max(maxch[:, g:g + 1], zg[g][:], axis=AX.X)
            nc.vector.reduce_sum(negsum_sb[:, s:s + 1], negparts[:], axis=AX.X)
            mxall = stat.tile([128, 1], F32)
            nc.vector.reduce_max(mxall[:], maxch[:], axis=AX.X)
            # thr = max_neg - BIG (pos entries hold z - BIG after fixup)
            nc.vector.tensor_scalar_add(thr_sb[:, s:s + 1], mxall[:], -BIG)

            msl = slice(256 * s, 256 * s + 256)
            sc1 = posp.tile([128, 256], F32, tag="sc1")
            nc.vector.scalar_tensor_tensor(
                out=sc1[:], in0=posgath[:], scalar=thr_sb[:, s:s + 1],
                in1=vm_sb[:, msl], op0=OP.is_gt, op1=OP.mult,
                accum_out=corr_sb[:, s:s + 1])
            sc2 = posp.tile([128, 256], F32, tag="sc2")
            nc.vector.scalar_tensor_tensor(
                out=sc2[:], in0=posgath[:], scalar=1.0,
                in1=wm_sb[:, msl], op0=OP.mult, op1=OP.mult,
                accum_out=possum_sb[:, s:s + 1])

        nc.sync.dma_start(negsum_d[:], negsum_sb[:])
        nc.sync.dma_start(thr_d[:], thr_sb[:])
        nc.sync.dma_start(possum_d[:], possum_sb[:])
        nc.sync.dma_start(corr_d[:], corr_sb[:])

    nc.compile()
    return nc


def _host_prep(feats1, feats2, overlap_inds):
    feats = np.concatenate([np.asarray(feats1, np.float32),
                            np.asarray(feats2, np.float32)], 0)
    featsT = np.ascontiguousarray(feats.T * np.float32(np.sqrt(TEMP)))
    ov = np.asarray(overlap_inds, bool)
    eye128 = np.eye(128, dtype=np.float32)

    in_maps = []
    wcnts, vcnts = [], []
    for c in range(NC):
        view2 = c >= 4
        cc = c - 4 if view2 else c
        self_s = 2048 + 512 * cc if view2 else 512 * cc
        other_s = 512 * cc if view2 else 2048 + 512 * cc
        keep = np.ones(N, bool)
        keep[self_s:self_s + 512] = False
        keep[other_s:other_s + 512] = False
        perm = np.concatenate([np.arange(self_s, self_s + 512),
                               np.arange(other_s, other_s + 512),
                               np.nonzero(keep)[0]])
        fT_c = np.ascontiguousarray(featsT[:, perm].astype(np.float16))

        V = np.zeros((128, NSTRIPE, 2, 128), np.float32)
        W = np.zeros((128, NSTRIPE, 2, 128), np.float32)
        ovfix = np.zeros((1, 512), np.float16)
        for s in range(NSTRIPE):
            for h in range(2):
                u = 2 * s + h
                m = 8 * cc + u
                rows = slice(64 * h, 64 * h + 64)
                lo = 64 * u - 128 * s
                V[rows, s, 0, lo:lo + 64] = 1.0
                W[rows, s, 0, lo:lo + 64] = 1.0
                if ov[m]:
                    V[rows, s, 1, lo:lo + 64] = 1.0
                    W[rows, s, 1, lo:lo + 64] = OTHER
                    ovfix[0, 64 * u:64 * u + 64] = -SQB
            V[:, s, 0, :] *= (1 - eye128)
            W[:, s, 0, :] *= (1 - eye128)

        wcnts.append(W.reshape(128, NSTRIPE, 256).sum(-1))
        vcnts.append(V.reshape(128, NSTRIPE, 256).sum(-1))
        in_maps.append({
            "featsT": fT_c,
            "vmask": np.ascontiguousarray(V.reshape(128, 1024)),
            "wmask": np.ascontiguousarray(W.reshape(128, 1024)),
            "ovfix": ovfix,
        })
    return in_maps, wcnts, vcnts


def kernel(feats1, feats2, overlap_inds, bs):
    assert int(bs) == BS
    feats1 = np.asarray(feats1, np.float32)
    feats2 = np.asarray(feats2, np.float32)
    assert feats1.shape == (N1, F) and feats2.shape == (N1, F)

    in_maps, wcnts, vcnts = _host_prep(feats1, feats2, overlap_inds)

    if "nc" not in _CACHE:
        _CACHE["nc"] = _build_nc()
    res = run_bass_kernel_spmd(_CACHE["nc"], in_maps, list(range(NC)))

    total_loss = 0.0
    total_corr = 0.0
    total_pos = 0.0
    for c in range(NC):
        out = res.results[c]
        negsum = out["negsum"].astype(np.float64)
        possum = out["possum"].astype(np.float64) + BIG * wcnts[c]
        corr = out["corr"].astype(np.float64)
        wcnt = wcnts[c].astype(np.float64)
        total_loss += (wcnt * np.log(negsum) - possum).sum()
        total_corr += corr.sum()
        total_pos += vcnts[c].sum(dtype=np.float64)

    loss = np.float32(total_loss / total_pos)
    acc = np.float32(total_corr / total_pos)
    return acc, loss


# revision 12
# speedup vs baseline: 1.2929x; 1.0133x over previous
"""Contrastive-loss kernel for 8 Trainium2 NeuronCores (SPMD, Bass/Tile).

Strategy (data-parallel over rows of the 4096x4096 similarity matrix):
  - Each core owns 512 rows (4 stripes of 128). It receives the full feature
    matrix, column-PERMUTED per core so its positive-pair blocks sit at
    program-constant offsets: perm = [own-view 512-block, other-view
    512-block, rest]. All core-dependence lives in input data (SPMD-safe).
  - featsT is pre-scaled by sqrt(TEMP) so PE matmuls produce logits directly.
  - Per stripe: two [128,2048] PSUM groups of matmuls (float32r, full rate);
    rank-1 bf16-free fixup matmuls subtract BIG=25 on same-class blocks so
    exp/max row-reductions see negatives only.
  - ACT: exp with fused row-accumulate -> neg_sum. DVE: strided
    tensor_tensor_reduce max chain -> max_neg (as thr = max_neg - BIG).
  - Positive blocks are read straight from PSUM (post-fixup, values z-BIG):
    fused compare+count (correct) and weighted-sum (pos logits) ops.
  - Host: tiny label math, final log/sum/divide on 4096-length vectors.
"""
import sys

if "/opt/trn_rl_repo" not in sys.path:
    sys.path.insert(0, "/opt/trn_rl_repo")

from contextlib import ExitStack

import numpy as np

import concourse.bass as bass
import concourse.tile as tile
from concourse import bacc, mybir
from concourse.bass_utils import run_bass_kernel_spmd

F32 = mybir.dt.float32
AX = mybir.AxisListType
OP = mybir.AluOpType
ACTF = mybir.ActivationFunctionType

K = 32
TEMP = 0.01
OTHER = 0.5
BS = 64
F = 128
N1 = 2048
N = 4096
NC = 8
RPC = 512          # rows per core
NSTRIPE = 4
BIG = 25.0
SQB = 5.0          # sqrt(BIG)

_CACHE: dict = {}


def _build_nc():
    nc = bacc.Bacc("TRN2", target_bir_lowering=False, debug=False, num_devices=NC)

    F16 = mybir.dt.float16
    fT_d = nc.dram_tensor("featsT", [F, N], F16, kind="ExternalInput").ap()
    vm_d = nc.dram_tensor("vmask", [128, 1024], F16, kind="ExternalInput").ap()
    wm_d = nc.dram_tensor("wmask", [128, 1024], F16, kind="ExternalInput").ap()
    ovf_d = nc.dram_tensor("ovfix", [1, 512], F16, kind="ExternalInput").ap()

    negsum_d = nc.dram_tensor("negsum", [128, 4], F32, kind="ExternalOutput").ap()
    thr_d = nc.dram_tensor("thr", [128, 4], F32, kind="ExternalOutput").ap()
    possum_d = nc.dram_tensor("possum", [128, 4], F32, kind="ExternalOutput").ap()
    corr_d = nc.dram_tensor("corr", [128, 4], F32, kind="ExternalOutput").ap()

    with tile.TileContext(nc) as tc, ExitStack() as ctx:
        singles = ctx.enter_context(tc.tile_pool(name="singles", bufs=1))
        psum = ctx.enter_context(tc.tile_pool(name="psum", bufs=2, space="PSUM"))
        expp = ctx.enter_context(tc.tile_pool(name="expp", bufs=2))
        maxp = ctx.enter_context(tc.tile_pool(name="maxp", bufs=2))
        posp = ctx.enter_context(tc.tile_pool(name="posp", bufs=2))
        stat = ctx.enter_context(tc.tile_pool(name="stat", bufs=3))
        outp = ctx.enter_context(tc.tile_pool(name="outs", bufs=1))

        fch = []
        for t in range(8):
            cht = singles.tile([F, 512], F16, name=f"fch{t}")
            nc.sync.dma_start(cht[:], fT_d[:, 512 * t:512 * (t + 1)])
            fch.append(cht)
        vm_sb = singles.tile([128, 1024], F16)
        nc.sync.dma_start(vm_sb[:], vm_d[:])
        wm_sb = singles.tile([128, 1024], F16)
        nc.sync.dma_start(wm_sb[:], wm_d[:])
        ovf_sb = singles.tile([1, 512], F16)
        nc.sync.dma_start(ovf_sb[:], ovf_d[:])
        ones_pos = singles.tile([1, 64], F16)
        nc.vector.memset(ones_pos[:], SQB)
        ones_neg = singles.tile([1, 64], F16)
        nc.vector.memset(ones_neg[:], -SQB)

        negsum_sb = outp.tile([128, 4], F32)
        thr_sb = outp.tile([128, 4], F32)
        possum_sb = outp.tile([128, 4], F32)
        corr_sb = outp.tile([128, 4], F32)

        for s in range(NSTRIPE):
            lhsT = fch[0][:, 128 * s:128 * s + 128]
            zg = [psum.tile([128, 2048], F32, tag="zg", name=f"zg{s}_{g}")
                  for g in range(2)]
            # all 8 big matmuls back-to-back with the same stationary lhsT
            for g in range(2):
                for t4 in range(4):
                    nc.tensor.matmul(
                        zg[g][:, 512 * t4:512 * (t4 + 1)],
                        lhsT,
                        fch[4 * g + t4][:],
                        start=True, stop=True)
            # fixups: subtract BIG on same-class blocks (group 0 only);
            # emitted after all big matmuls so PE switches weights only once.
            for h in range(2):
                u = 2 * s + h
                nc.tensor.matmul(
                    zg[0][64 * h:64 * h + 64, 64 * u:64 * u + 64],
                    ones_pos[:], ones_neg[:],
                    start=False, stop=True, skip_group_check=True)
                nc.tensor.matmul(
                    zg[0][64 * h:64 * h + 64, 512 + 64 * u:512 + 64 * u + 64],
                    ones_pos[:], ovf_sb[:, 64 * u:64 * u + 64],
                    start=False, stop=True, skip_group_check=True)

            # copy the two positive-block regions out of PSUM early (ACT) so
            # the PSUM slot is released after exp+max, not after the pos ops
            posgath = posp.tile([128, 256], F32, tag="posgath")
            for b in range(2):
                nc.scalar.copy(
                    posgath[:, 128 * b:128 * b + 128],
                    zg[0][:, 512 * b + 128 * s:512 * b + 128 * s + 128])

            negparts = stat.tile([128, 2], F32)
            maxch = stat.tile([128, 2], F32)
            for g in range(2):
                ex = expp.tile([128, 2048], F32, tag="ex", name=f"ex{s}_{g}")
                nc.scalar.activation(ex[:], zg[g][:], ACTF.Exp,
                                     accum_out=negparts[:, g:g + 1])
                nc.vector.reduce_max(maxch[:, g:g + 1], zg[g][:], axis=AX.X)
            nc.vector.reduce_sum(negsum_sb[:, s:s + 1], negparts[:], axis=AX.X)
            mxall = stat.tile([128, 1], F32)
            nc.vector.reduce_max(mxall[:], maxch[:], axis=AX.X)
            # thr = max_neg - BIG (pos entries hold z - BIG after fixup)
            nc.vector.tensor_scalar_add(thr_sb[:, s:s + 1], mxall[:], -BIG)

            msl = slice(256 * s, 256 * s + 256)
            sc1 = posp.tile([128, 256], F32, tag="sc1")
            nc.vector.scalar_tensor_tensor(
                out=sc1[:], in0=posgath[:], scalar=thr_sb[:, s:s + 1],
                in1=vm_sb[:, msl], op0=OP.is_gt, op1=OP.mult,
                accum_out=corr_sb[:, s:s + 1])
            sc2 = posp.tile([128, 256], F32, tag="sc2")
            nc.vector.scalar_tensor_tensor(
                out=sc2[:], in0=posgath[:], scalar=1.0,
                in1=wm_sb[:, msl], op0=OP.mult, op1=OP.mult,
                accum_out=possum_sb[:, s:s + 1])

        nc.sync.dma_start(negsum_d[:], negsum_sb[:])
        nc.sync.dma_start(thr_d[:], thr_sb[:])
        nc.sync.dma_start(possum_d[:], possum_sb[:])
        nc.sync.dma_start(corr_d[:], corr_sb[:])

    nc.compile()
    return nc


def _host_prep(feats1, feats2, overlap_inds):
    feats = np.concatenate([np.asarray(feats1, np.float32),
                            np.asarray(feats2, np.float32)], 0)
    featsT = np.ascontiguousarray(feats.T * np.float32(np.sqrt(TEMP)))
    ov = np.asarray(overlap_inds, bool)
    eye128 = np.eye(128, dtype=np.float32)

    in_maps = []
    wcnts, vcnts = [], []
    for c in range(NC):
        view2 = c >= 4
        cc = c - 4 if view2 else c
        self_s = 2048 + 512 * cc if view2 else 512 * cc
        other_s = 512 * cc if view2 else 2048 + 512 * cc
        keep = np.ones(N, bool)
        keep[self_s:self_s + 512] = False
        keep[other_s:other_s + 512] = False
        perm = np.concatenate([np.arange(self_s, self_s + 512),
                               np.arange(other_s, other_s + 512),
                               np.nonzero(keep)[0]])
        fT_c = np.ascontiguousarray(featsT[:, perm].astype(np.float16))

        V = np.zeros((128, NSTRIPE, 2, 128), np.float32)
        W = np.zeros((128, NSTRIPE, 2, 128), np.float32)
        ovfix = np.zeros((1, 512), np.float16)
        for s in range(NSTRIPE):
            for h in range(2):
                u = 2 * s + h
                m = 8 * cc + u
                rows = slice(64 * h, 64 * h + 64)
                lo = 64 * u - 128 * s
                V[rows, s, 0, lo:lo + 64] = 1.0
                W[rows, s, 0, lo:lo + 64] = 1.0
                if ov[m]:
                    V[rows, s, 1, lo:lo + 64] = 1.0
                    W[rows, s, 1, lo:lo + 64] = OTHER
                    ovfix[0, 64 * u:64 * u + 64] = -SQB
            V[:, s, 0, :] *= (1 - eye128)
            W[:, s, 0, :] *= (1 - eye128)

        wcnts.append(W.reshape(128, NSTRIPE, 256).sum(-1))
        vcnts.append(V.reshape(128, NSTRIPE, 256).sum(-1))
        in_maps.append({
            "featsT": fT_c,
            "vmask": np.ascontiguousarray(V.reshape(128, 1024).astype(np.float16)),
            "wmask": np.ascontiguousarray(W.reshape(128, 1024).astype(np.float16)),
            "ovfix": ovfix,
        })
    return in_maps, wcnts, vcnts


def kernel(feats1, feats2, overlap_inds, bs):
    assert int(bs) == BS
    feats1 = np.asarray(feats1, np.float32)
    feats2 = np.asarray(feats2, np.float32)
    assert feats1.shape == (N1, F) and feats2.shape == (N1, F)

    in_maps, wcnts, vcnts = _host_prep(feats1, feats2, overlap_inds)

    if "nc" not in _CACHE:
        _CACHE["nc"] = _build_nc()
    res = run_bass_kernel_spmd(_CACHE["nc"], in_maps, list(range(NC)))

    total_loss = 0.0
    total_corr = 0.0
    total_pos = 0.0
    for c in range(NC):
        out = res.results[c]
        negsum = out["negsum"].astype(np.float64)
        possum = out["possum"].astype(np.float64) + BIG * wcnts[c]
        corr = out["corr"].astype(np.float64)
        wcnt = wcnts[c].astype(np.float64)
        total_loss += (wcnt * np.log(negsum) - possum).sum()
        total_corr += corr.sum()
        total_pos += vcnts[c].sum(dtype=np.float64)

    loss = np.float32(total_loss / total_pos)
    acc = np.float32(total_corr / total_pos)
    return acc, loss


# revision 15
# speedup vs baseline: 1.3886x; 1.0740x over previous
"""Contrastive-loss kernel for 8 Trainium2 NeuronCores (SPMD, Bass/Tile).

Strategy (data-parallel over rows of the 4096x4096 similarity matrix):
  - Each core owns 512 rows (4 stripes of 128). It receives the full feature
    matrix, column-PERMUTED per core so its positive-pair blocks sit at
    program-constant offsets: perm = [own-view 512-block, other-view
    512-block, rest]. All core-dependence lives in input data (SPMD-safe).
  - featsT is pre-scaled by sqrt(TEMP) so PE matmuls produce logits directly.
  - Per stripe: two [128,2048] PSUM groups of matmuls (float32r, full rate);
    rank-1 bf16-free fixup matmuls subtract BIG=25 on same-class blocks so
    exp/max row-reductions see negatives only.
  - ACT: exp with fused row-accumulate -> neg_sum. DVE: strided
    tensor_tensor_reduce max chain -> max_neg (as thr = max_neg - BIG).
  - Positive blocks are read straight from PSUM (post-fixup, values z-BIG):
    fused compare+count (correct) and weighted-sum (pos logits) ops.
  - Host: tiny label math, final log/sum/divide on 4096-length vectors.
"""
import sys

if "/opt/trn_rl_repo" not in sys.path:
    sys.path.insert(0, "/opt/trn_rl_repo")

from contextlib import ExitStack

import numpy as np

import concourse.bass as bass
import concourse.tile as tile
from concourse import bacc, mybir
from concourse.bass_utils import run_bass_kernel_spmd

F32 = mybir.dt.float32
AX = mybir.AxisListType
OP = mybir.AluOpType
ACTF = mybir.ActivationFunctionType

K = 32
TEMP = 0.01
OTHER = 0.5
BS = 64
F = 128
N1 = 2048
N = 4096
NC = 8
RPC = 512          # rows per core
NSTRIPE = 4
BIG = 25.0
SQB = 5.0          # sqrt(BIG)

_CACHE: dict = {}


def _build_nc():
    nc = bacc.Bacc("TRN2", target_bir_lowering=False, debug=False, num_devices=NC)

    F16 = mybir.dt.float16
    fT_d = nc.dram_tensor("featsT", [F, N], F16, kind="ExternalInput").ap()
    vm_d = nc.dram_tensor("vmask", [128, 1024], F16, kind="ExternalInput").ap()
    wm_d = nc.dram_tensor("wmask", [128, 1024], F16, kind="ExternalInput").ap()
    ovf_d = nc.dram_tensor("ovfix", [1, 512], F16, kind="ExternalInput").ap()

    negsum_d = nc.dram_tensor("negsum", [128, 4], F32, kind="ExternalOutput").ap()
    thr_d = nc.dram_tensor("thr", [128, 4], F32, kind="ExternalOutput").ap()
    possum_d = nc.dram_tensor("possum", [128, 4], F32, kind="ExternalOutput").ap()
    corr_d = nc.dram_tensor("corr", [128, 4], F32, kind="ExternalOutput").ap()

    with tile.TileContext(nc) as tc, ExitStack() as ctx:
        singles = ctx.enter_context(tc.tile_pool(name="singles", bufs=1))
        expp = ctx.enter_context(tc.tile_pool(name="expp", bufs=2))
        posp = ctx.enter_context(tc.tile_pool(name="posp", bufs=2))
        stat = ctx.enter_context(tc.tile_pool(name="stat", bufs=3))
        outp = ctx.enter_context(tc.tile_pool(name="outs", bufs=1))

        fch = []
        for t in range(8):
            cht = singles.tile([F, 512], F16, name=f"fch{t}")
            eng = nc.sync if t < 4 else nc.gpsimd
            eng.dma_start(cht[:], fT_d[:, 512 * t:512 * (t + 1)])
            fch.append(cht)
        vm_sb = singles.tile([128, 1024], F16)
        nc.sync.dma_start(vm_sb[:], vm_d[:])
        wm_sb = singles.tile([128, 1024], F16)
        nc.gpsimd.dma_start(wm_sb[:], wm_d[:])
        ovf_sb = singles.tile([1, 512], F16)
        nc.gpsimd.dma_start(ovf_sb[:], ovf_d[:])
        ones_pos = singles.tile([1, 64], F16)
        nc.vector.memset(ones_pos[:], SQB)
        ones_neg = singles.tile([1, 64], F16)
        nc.vector.memset(ones_neg[:], -SQB)

        negsum_sb = outp.tile([128, 4], F32)
        thr_sb = outp.tile([128, 4], F32)
        possum_sb = outp.tile([128, 4], F32)
        corr_sb = outp.tile([128, 4], F32)

        # ---- pre-pass: positive blocks as plain-z matmuls (runs in the
        # DMA shadow; the temp PSUM pool closes before the main loop) ----
        posgath = singles.tile([128, 1024], F32)
        with tc.tile_pool(name="pspos", bufs=1, space="PSUM") as pspos:
            pz = pspos.tile([128, 1024], F32)
            for s in range(NSTRIPE):
                for b in range(2):
                    nc.tensor.matmul(
                        pz[:, 256 * s + 128 * b:256 * s + 128 * b + 128],
                        fch[0][:, 128 * s:128 * s + 128],
                        fch[b][:, 128 * s:128 * s + 128],
                        start=True, stop=True)
            nc.scalar.copy(posgath[:], pz[:])

        psum = ctx.enter_context(tc.tile_pool(name="psum", bufs=2, space="PSUM"))
        for s in range(NSTRIPE):
            lhsT = fch[0][:, 128 * s:128 * s + 128]
            zg = [psum.tile([128, 2048], F32, tag="zg", name=f"zg{s}_{g}")
                  for g in range(2)]
            # all 8 big matmuls back-to-back with the same stationary lhsT
            for g in range(2):
                for t4 in range(4):
                    nc.tensor.matmul(
                        zg[g][:, 512 * t4:512 * (t4 + 1)],
                        lhsT,
                        fch[4 * g + t4][:],
                        start=True, stop=True)
            # fixups: subtract BIG on same-class blocks (group 0 only);
            # emitted after all big matmuls so PE switches weights only once.
            for h in range(2):
                u = 2 * s + h
                nc.tensor.matmul(
                    zg[0][64 * h:64 * h + 64, 64 * u:64 * u + 64],
                    ones_pos[:], ones_neg[:],
                    start=False, stop=True, skip_group_check=True)
                nc.tensor.matmul(
                    zg[0][64 * h:64 * h + 64, 512 + 64 * u:512 + 64 * u + 64],
                    ones_pos[:], ovf_sb[:, 64 * u:64 * u + 64],
                    start=False, stop=True, skip_group_check=True)

            negparts = stat.tile([128, 2], F32)
            maxch = stat.tile([128, 2], F32)
            for g in range(2):
                ex = expp.tile([128, 2048], F32, tag="ex", name=f"ex{s}_{g}")
                nc.scalar.activation(ex[:], zg[g][:], ACTF.Exp,
                                     accum_out=negparts[:, g:g + 1])
                nc.vector.reduce_max(maxch[:, g:g + 1], zg[g][:], axis=AX.X)
            nc.vector.reduce_sum(negsum_sb[:, s:s + 1], negparts[:], axis=AX.X)
            # thr = max_neg directly (pos blocks hold plain z from pre-pass)
            nc.vector.reduce_max(thr_sb[:, s:s + 1], maxch[:], axis=AX.X)

            msl = slice(256 * s, 256 * s + 256)
            sc1 = posp.tile([128, 256], F32, tag="sc1")
            nc.vector.scalar_tensor_tensor(
                out=sc1[:], in0=posgath[:, msl], scalar=thr_sb[:, s:s + 1],
                in1=vm_sb[:, msl], op0=OP.is_gt, op1=OP.mult,
                accum_out=corr_sb[:, s:s + 1])
            sc2 = posp.tile([128, 256], F32, tag="sc2")
            nc.vector.scalar_tensor_tensor(
                out=sc2[:], in0=posgath[:, msl], scalar=1.0,
                in1=wm_sb[:, msl], op0=OP.mult, op1=OP.mult,
                accum_out=possum_sb[:, s:s + 1])

        nc.sync.dma_start(negsum_d[:], negsum_sb[:])
        nc.sync.dma_start(thr_d[:], thr_sb[:])
        nc.sync.dma_start(possum_d[:], possum_sb[:])
        nc.sync.dma_start(corr_d[:], corr_sb[:])

    nc.compile()
    return nc


def _host_prep(feats1, feats2, overlap_inds):
    feats = np.concatenate([np.asarray(feats1, np.float32),
                            np.asarray(feats2, np.float32)], 0)
    featsT = np.ascontiguousarray(feats.T * np.float32(np.sqrt(TEMP)))
    ov = np.asarray(overlap_inds, bool)
    eye128 = np.eye(128, dtype=np.float32)

    in_maps = []
    wcnts, vcnts = [], []
    for c in range(NC):
        view2 = c >= 4
        cc = c - 4 if view2 else c
        self_s = 2048 + 512 * cc if view2 else 512 * cc
        other_s = 512 * cc if view2 else 2048 + 512 * cc
        keep = np.ones(N, bool)
        keep[self_s:self_s + 512] = False
        keep[other_s:other_s + 512] = False
        perm = np.concatenate([np.arange(self_s, self_s + 512),
                               np.arange(other_s, other_s + 512),
                               np.nonzero(keep)[0]])
        fT_c = np.ascontiguousarray(featsT[:, perm].astype(np.float16))

        V = np.zeros((128, NSTRIPE, 2, 128), np.float32)
        W = np.zeros((128, NSTRIPE, 2, 128), np.float32)
        ovfix = np.zeros((1, 512), np.float16)
        for s in range(NSTRIPE):
            for h in range(2):
                u = 2 * s + h
                m = 8 * cc + u
                rows = slice(64 * h, 64 * h + 64)
                lo = 64 * u - 128 * s
                V[rows, s, 0, lo:lo + 64] = 1.0
                W[rows, s, 0, lo:lo + 64] = 1.0
                if ov[m]:
                    V[rows, s, 1, lo:lo + 64] = 1.0
                    W[rows, s, 1, lo:lo + 64] = OTHER
                    ovfix[0, 64 * u:64 * u + 64] = -SQB
            V[:, s, 0, :] *= (1 - eye128)
            W[:, s, 0, :] *= (1 - eye128)

        wcnts.append(W.reshape(128, NSTRIPE, 256).sum(-1))
        vcnts.append(V.reshape(128, NSTRIPE, 256).sum(-1))
        in_maps.append({
            "featsT": fT_c,
            "vmask": np.ascontiguousarray(V.reshape(128, 1024).astype(np.float16)),
            "wmask": np.ascontiguousarray(W.reshape(128, 1024).astype(np.float16)),
            "ovfix": ovfix,
        })
    return in_maps, wcnts, vcnts


def kernel(feats1, feats2, overlap_inds, bs):
    assert int(bs) == BS
    feats1 = np.asarray(feats1, np.float32)
    feats2 = np.asarray(feats2, np.float32)
    assert feats1.shape == (N1, F) and feats2.shape == (N1, F)

    in_maps, wcnts, vcnts = _host_prep(feats1, feats2, overlap_inds)

    if "nc" not in _CACHE:
        _CACHE["nc"] = _build_nc()
    res = run_bass_kernel_spmd(_CACHE["nc"], in_maps, list(range(NC)))

    total_loss = 0.0
    total_corr = 0.0
    total_pos = 0.0
    for c in range(NC):
        out = res.results[c]
        negsum = out["negsum"].astype(np.float64)
        possum = out["possum"].astype(np.float64)
        corr = out["corr"].astype(np.float64)
        wcnt = wcnts[c].astype(np.float64)
        total_loss += (wcnt * np.log(negsum) - possum).sum()
        total_corr += corr.sum()
        total_pos += vcnts[c].sum(dtype=np.float64)

    loss = np.float32(total_loss / total_pos)
    acc = np.float32(total_corr / total_pos)
    return acc, loss


# revision 18
# speedup vs baseline: 1.5440x; 1.1119x over previous
"""Contrastive-loss kernel for 8 Trainium2 NeuronCores (SPMD, Bass/Tile).

Strategy (data-parallel over rows of the 4096x4096 similarity matrix):
  - Each core owns 512 rows (4 stripes of 128). It receives the full feature
    matrix, column-PERMUTED per core so its positive-pair blocks sit at
    program-constant offsets: perm = [own-view 512-block, other-view
    512-block, rest]. All core-dependence lives in input data (SPMD-safe).
  - featsT is pre-scaled by sqrt(TEMP) so PE matmuls produce logits directly.
  - Per stripe: two [128,2048] PSUM groups of matmuls (float32r, full rate);
    rank-1 bf16-free fixup matmuls subtract BIG=25 on same-class blocks so
    exp/max row-reductions see negatives only.
  - ACT: exp with fused row-accumulate -> neg_sum. DVE: strided
    tensor_tensor_reduce max chain -> max_neg (as thr = max_neg - BIG).
  - Positive blocks are read straight from PSUM (post-fixup, values z-BIG):
    fused compare+count (correct) and weighted-sum (pos logits) ops.
  - Host: tiny label math, final log/sum/divide on 4096-length vectors.
"""
import sys

if "/opt/trn_rl_repo" not in sys.path:
    sys.path.insert(0, "/opt/trn_rl_repo")

from contextlib import ExitStack

import numpy as np

import concourse.bass as bass
import concourse.tile as tile
from concourse import bacc, mybir
from concourse.bass_utils import run_bass_kernel_spmd

F32 = mybir.dt.float32
AX = mybir.AxisListType
OP = mybir.AluOpType
ACTF = mybir.ActivationFunctionType

K = 32
TEMP = 0.01
OTHER = 0.5
BS = 64
F = 128
N1 = 2048
N = 4096
NC = 8
RPC = 512          # rows per core
NSTRIPE = 4
BIG = 25.0
SQB = 5.0          # sqrt(BIG)

_CACHE: dict = {}


def _build_nc():
    nc = bacc.Bacc("TRN2", target_bir_lowering=False, debug=False, num_devices=NC)

    F16 = mybir.dt.float16
    fT_d = nc.dram_tensor("featsT", [F, N], F16, kind="ExternalInput").ap()
    vm_d = nc.dram_tensor("vmask", [128, 1024], F16, kind="ExternalInput").ap()
    wm_d = nc.dram_tensor("wmask", [128, 1024], F16, kind="ExternalInput").ap()
    ovf_d = nc.dram_tensor("ovfix", [1, 512], F16, kind="ExternalInput").ap()

    negsum_d = nc.dram_tensor("negsum", [128, 4], F32, kind="ExternalOutput").ap()
    thr_d = nc.dram_tensor("thr", [128, 4], F32, kind="ExternalOutput").ap()
    possum_d = nc.dram_tensor("possum", [128, 4], F32, kind="ExternalOutput").ap()
    corr_d = nc.dram_tensor("corr", [128, 4], F32, kind="ExternalOutput").ap()

    with tile.TileContext(nc) as tc, ExitStack() as ctx:
        singles = ctx.enter_context(tc.tile_pool(name="singles", bufs=1))
        expp = ctx.enter_context(tc.tile_pool(name="expp", bufs=2))
        posp = ctx.enter_context(tc.tile_pool(name="posp", bufs=2))
        stat = ctx.enter_context(tc.tile_pool(name="stat", bufs=3))
        outp = ctx.enter_context(tc.tile_pool(name="outs", bufs=1))

        fch = []
        dma_eng = [nc.sync, nc.scalar, nc.sync, nc.scalar,
                   nc.gpsimd, nc.gpsimd, nc.sync, nc.gpsimd]
        for t in range(8):
            cht = singles.tile([F, 512], F16, name=f"fch{t}")
            dma_eng[t].dma_start(cht[:], fT_d[:, 512 * t:512 * (t + 1)])
            fch.append(cht)
        vm_sb = singles.tile([128, 1024], F16)
        nc.sync.dma_start(vm_sb[:], vm_d[:])
        wm_sb = singles.tile([128, 1024], F16)
        nc.gpsimd.dma_start(wm_sb[:], wm_d[:])
        ovf_sb = singles.tile([1, 512], F16)
        nc.gpsimd.dma_start(ovf_sb[:], ovf_d[:])
        ones_pos = singles.tile([1, 64], F16)
        nc.vector.memset(ones_pos[:], SQB)
        ones_neg = singles.tile([1, 64], F16)
        nc.vector.memset(ones_neg[:], -SQB)

        negsum_sb = outp.tile([128, 4], F32)
        thr_sb = outp.tile([128, 4], F32)
        possum_sb = outp.tile([128, 4], F32)
        corr_sb = outp.tile([128, 4], F32)

        # ---- pre-pass: positive blocks as plain-z matmuls (runs in the
        # DMA shadow; the temp PSUM pool closes before the main loop) ----
        posgath = singles.tile([128, 1024], F32)
        with tc.tile_pool(name="pspos", bufs=1, space="PSUM") as pspos:
            pz = pspos.tile([128, 1024], F32)
            for s in range(NSTRIPE):
                for b in range(2):
                    nc.tensor.matmul(
                        pz[:, 256 * s + 128 * b:256 * s + 128 * b + 128],
                        fch[0][:, 128 * s:128 * s + 128],
                        fch[b][:, 128 * s:128 * s + 128],
                        start=True, stop=True)
            nc.scalar.copy(posgath[:], pz[:])

        psum = ctx.enter_context(tc.tile_pool(name="psum", bufs=4, space="PSUM"))
        for s in range(NSTRIPE):
            lhsT = fch[0][:, 128 * s:128 * s + 128]
            zg = [psum.tile([128, 1024], F32, tag="zg", name=f"zg{s}_{g}")
                  for g in range(4)]
            # all 8 big matmuls back-to-back with the same stationary lhsT
            for g in range(4):
                for t2 in range(2):
                    nc.tensor.matmul(
                        zg[g][:, 512 * t2:512 * (t2 + 1)],
                        lhsT,
                        fch[2 * g + t2][:],
                        start=True, stop=True)
            # fixups: subtract BIG on same-class blocks (group 0 only);
            # emitted after all big matmuls so PE switches weights only once.
            for h in range(2):
                u = 2 * s + h
                nc.tensor.matmul(
                    zg[0][64 * h:64 * h + 64, 64 * u:64 * u + 64],
                    ones_pos[:], ones_neg[:],
                    start=False, stop=True, skip_group_check=True)
                nc.tensor.matmul(
                    zg[0][64 * h:64 * h + 64, 512 + 64 * u:512 + 64 * u + 64],
                    ones_pos[:], ovf_sb[:, 64 * u:64 * u + 64],
                    start=False, stop=True, skip_group_check=True)

            negparts = stat.tile([128, 4], F32)
            maxch = stat.tile([128, 4], F32)
            for g in range(4):
                ex = expp.tile([128, 1024], F32, tag="ex", name=f"ex{s}_{g}")
                nc.scalar.activation(ex[:], zg[g][:], ACTF.Exp,
                                     accum_out=negparts[:, g:g + 1])
                nc.vector.reduce_max(maxch[:, g:g + 1], zg[g][:], axis=AX.X)
            nc.vector.reduce_sum(negsum_sb[:, s:s + 1], negparts[:], axis=AX.X)
            # thr = max_neg directly (pos blocks hold plain z from pre-pass)
            nc.vector.reduce_max(thr_sb[:, s:s + 1], maxch[:], axis=AX.X)

            msl = slice(256 * s, 256 * s + 256)
            sc1 = posp.tile([128, 256], F32, tag="sc1")
            nc.vector.scalar_tensor_tensor(
                out=sc1[:], in0=posgath[:, msl], scalar=thr_sb[:, s:s + 1],
                in1=vm_sb[:, msl], op0=OP.is_gt, op1=OP.mult,
                accum_out=corr_sb[:, s:s + 1])
            sc2 = posp.tile([128, 256], F32, tag="sc2")
            nc.vector.scalar_tensor_tensor(
                out=sc2[:], in0=posgath[:, msl], scalar=1.0,
                in1=wm_sb[:, msl], op0=OP.mult, op1=OP.mult,
                accum_out=possum_sb[:, s:s + 1])

        nc.sync.dma_start(negsum_d[:], negsum_sb[:])
        nc.sync.dma_start(thr_d[:], thr_sb[:])
        nc.sync.dma_start(possum_d[:], possum_sb[:])
        nc.sync.dma_start(corr_d[:], corr_sb[:])

    nc.compile()
    return nc


def _host_prep(feats1, feats2, overlap_inds):
    feats = np.concatenate([np.asarray(feats1, np.float32),
                            np.asarray(feats2, np.float32)], 0)
    featsT = np.ascontiguousarray(feats.T * np.float32(np.sqrt(TEMP)))
    ov = np.asarray(overlap_inds, bool)
    eye128 = np.eye(128, dtype=np.float32)

    in_maps = []
    wcnts, vcnts = [], []
    for c in range(NC):
        view2 = c >= 4
        cc = c - 4 if view2 else c
        self_s = 2048 + 512 * cc if view2 else 512 * cc
        other_s = 512 * cc if view2 else 2048 + 512 * cc
        keep = np.ones(N, bool)
        keep[self_s:self_s + 512] = False
        keep[other_s:other_s + 512] = False
        perm = np.concatenate([np.arange(self_s, self_s + 512),
                               np.arange(other_s, other_s + 512),
                               np.nonzero(keep)[0]])
        fT_c = np.ascontiguousarray(featsT[:, perm].astype(np.float16))

        V = np.zeros((128, NSTRIPE, 2, 128), np.float32)
        W = np.zeros((128, NSTRIPE, 2, 128), np.float32)
        ovfix = np.zeros((1, 512), np.float16)
        for s in range(NSTRIPE):
            for h in range(2):
                u = 2 * s + h
                m = 8 * cc + u
                rows = slice(64 * h, 64 * h + 64)
                lo = 64 * u - 128 * s
                V[rows, s, 0, lo:lo + 64] = 1.0
                W[rows, s, 0, lo:lo + 64] = 1.0
                if ov[m]:
                    V[rows, s, 1, lo:lo + 64] = 1.0
                    W[rows, s, 1, lo:lo + 64] = OTHER
                    ovfix[0, 64 * u:64 * u + 64] = -SQB
            V[:, s, 0, :] *= (1 - eye128)
            W[:, s, 0, :] *= (1 - eye128)

        wcnts.append(W.reshape(128, NSTRIPE, 256).sum(-1))
        vcnts.append(V.reshape(128, NSTRIPE, 256).sum(-1))
        in_maps.append({
            "featsT": fT_c,
            "vmask": np.ascontiguousarray(V.reshape(128, 1024).astype(np.float16)),
            "wmask": np.ascontiguousarray(W.reshape(128, 1024).astype(np.float16)),
            "ovfix": ovfix,
        })
    return in_maps, wcnts, vcnts


def kernel(feats1, feats2, overlap_inds, bs):
    assert int(bs) == BS
    feats1 = np.asarray(feats1, np.float32)
    feats2 = np.asarray(feats2, np.float32)
    assert feats1.shape == (N1, F) and feats2.shape == (N1, F)

    in_maps, wcnts, vcnts = _host_prep(feats1, feats2, overlap_inds)

    if "nc" not in _CACHE:
        _CACHE["nc"] = _build_nc()
    res = run_bass_kernel_spmd(_CACHE["nc"], in_maps, list(range(NC)))

    total_loss = 0.0
    total_corr = 0.0
    total_pos = 0.0
    for c in range(NC):
        out = res.results[c]
        negsum = out["negsum"].astype(np.float64)
        possum = out["possum"].astype(np.float64)
        corr = out["corr"].astype(np.float64)
        wcnt = wcnts[c].astype(np.float64)
        total_loss += (wcnt * np.log(negsum) - possum).sum()
        total_corr += corr.sum()
        total_pos += vcnts[c].sum(dtype=np.float64)

    loss = np.float32(total_loss / total_pos)
    acc = np.float32(total_corr / total_pos)
    return acc, loss


# revision 22
# speedup vs baseline: 1.6779x; 1.0867x over previous
"""Contrastive-loss kernel for 8 Trainium2 NeuronCores (SPMD, Bass/Tile).

Strategy (data-parallel over rows of the 4096x4096 similarity matrix):
  - Each core owns 512 rows (4 stripes of 128). It receives the full feature
    matrix, column-PERMUTED per core so its positive-pair blocks sit at
    program-constant offsets: perm = [own-view 512-block, other-view
    512-block, rest]. All core-dependence lives in input data (SPMD-safe).
  - featsT is pre-scaled by sqrt(TEMP) so PE matmuls produce logits directly.
  - Per stripe: two [128,2048] PSUM groups of matmuls (float32r, full rate);
    rank-1 bf16-free fixup matmuls subtract BIG=25 on same-class blocks so
    exp/max row-reductions see negatives only.
  - ACT: exp with fused row-accumulate -> neg_sum. DVE: strided
    tensor_tensor_reduce max chain -> max_neg (as thr = max_neg - BIG).
  - Positive blocks are read straight from PSUM (post-fixup, values z-BIG):
    fused compare+count (correct) and weighted-sum (pos logits) ops.
  - Host: tiny label math, final log/sum/divide on 4096-length vectors.
"""
import sys

if "/opt/trn_rl_repo" not in sys.path:
    sys.path.insert(0, "/opt/trn_rl_repo")

from contextlib import ExitStack

import numpy as np

import concourse.bass as bass
import concourse.tile as tile
from concourse import bacc, mybir
from concourse.bass_utils import run_bass_kernel_spmd

F32 = mybir.dt.float32
AX = mybir.AxisListType
OP = mybir.AluOpType
ACTF = mybir.ActivationFunctionType

K = 32
TEMP = 0.01
OTHER = 0.5
BS = 64
F = 128
N1 = 2048
N = 4096
NC = 8
RPC = 512          # rows per core
NSTRIPE = 4
BIG = 25.0
SQB = 5.0          # sqrt(BIG)

_CACHE: dict = {}


def _build_nc():
    nc = bacc.Bacc("TRN2", target_bir_lowering=False, debug=False, num_devices=NC)

    F16 = mybir.dt.float16
    fT_d = nc.dram_tensor("featsT", [8, F, 512], F16, kind="ExternalInput").ap()
    vm_d = nc.dram_tensor("vmask", [128, 1024], F16, kind="ExternalInput").ap()
    wm_d = nc.dram_tensor("wmask", [128, 1024], F16, kind="ExternalInput").ap()
    ovf_d = nc.dram_tensor("ovfix", [1, 512], F16, kind="ExternalInput").ap()

    out_d = nc.dram_tensor("outs", [128, 16], F32, kind="ExternalOutput").ap()

    with tile.TileContext(nc) as tc, ExitStack() as ctx:
        singles = ctx.enter_context(tc.tile_pool(name="singles", bufs=1))
        expp = ctx.enter_context(tc.tile_pool(name="expp", bufs=2))
        posp = ctx.enter_context(tc.tile_pool(name="posp", bufs=2))
        stat = ctx.enter_context(tc.tile_pool(name="stat", bufs=3))
        outp = ctx.enter_context(tc.tile_pool(name="outs", bufs=1))

        ovf_sb = singles.tile([1, 512], F16)
        nc.gpsimd.dma_start(ovf_sb[:], ovf_d[:])
        fch = []
        dma_eng = [nc.sync, nc.scalar, nc.sync, nc.scalar,
                   nc.gpsimd, nc.gpsimd, nc.sync, nc.gpsimd]
        for t in range(8):
            cht = singles.tile([F, 512], F16, name=f"fch{t}")
            dma_eng[t].dma_start(cht[:], fT_d[t])
            fch.append(cht)
        vm_sb = singles.tile([128, 1024], F16)
        nc.sync.dma_start(vm_sb[:], vm_d[:])
        wm_sb = singles.tile([128, 1024], F16)
        nc.gpsimd.dma_start(wm_sb[:], wm_d[:])
        ones_pos = singles.tile([1, 64], F16)
        nc.vector.memset(ones_pos[:], SQB)
        ones_neg = singles.tile([1, 64], F16)
        nc.vector.memset(ones_neg[:], -SQB)

        out_sb = outp.tile([128, 16], F32)
        negsum_sb = out_sb[:, 0:4]
        thr_sb = out_sb[:, 4:8]
        possum_sb = out_sb[:, 8:12]
        corr_sb = out_sb[:, 12:16]

        # ---- pre-pass: positive blocks as plain-z matmuls (runs in the
        # DMA shadow; shares the main PSUM pool's slot rotation) ----
        psum = ctx.enter_context(tc.tile_pool(name="psum", bufs=4, space="PSUM"))
        posgath = singles.tile([128, 1024], F32)
        pz = psum.tile([128, 1024], F32, tag="zg", name="pz")
        for s in range(NSTRIPE):
            for b in range(2):
                nc.tensor.matmul(
                    pz[:, 256 * s + 128 * b:256 * s + 128 * b + 128],
                    fch[0][:, 128 * s:128 * s + 128],
                    fch[b][:, 128 * s:128 * s + 128],
                    start=True, stop=True)
        nc.scalar.copy(posgath[:], pz[:])
        for s in range(NSTRIPE):
            lhsT = fch[0][:, 128 * s:128 * s + 128]
            zg = [psum.tile([128, 1024], F32, tag="zg", name=f"zg{s}_{g}")
                  for g in range(4)]
            # all 8 big matmuls back-to-back with the same stationary lhsT
            for g in range(4):
                for t2 in range(2):
                    nc.tensor.matmul(
                        zg[g][:, 512 * t2:512 * (t2 + 1)],
                        lhsT,
                        fch[2 * g + t2][:],
                        start=True, stop=True)
            # fixups: subtract BIG on same-class blocks (group 0 only);
            # emitted after all big matmuls so PE switches weights only once.
            for h in range(2):
                u = 2 * s + h
                nc.tensor.matmul(
                    zg[0][64 * h:64 * h + 64, 64 * u:64 * u + 64],
                    ones_pos[:], ones_neg[:],
                    start=False, stop=True, skip_group_check=True)
                nc.tensor.matmul(
                    zg[0][64 * h:64 * h + 64, 512 + 64 * u:512 + 64 * u + 64],
                    ones_pos[:], ovf_sb[:, 64 * u:64 * u + 64],
                    start=False, stop=True, skip_group_check=True)

            negparts = stat.tile([128, 4], F32)
            maxch = stat.tile([128, 4], F32)
            for g in range(4):
                ex = expp.tile([128, 1024], F32, tag="ex", name=f"ex{s}_{g}")
                nc.scalar.activation(ex[:], zg[g][:], ACTF.Exp,
                                     accum_out=negparts[:, g:g + 1])
                nc.vector.reduce_max(maxch[:, g:g + 1], zg[g][:], axis=AX.X)
            nc.vector.reduce_sum(negsum_sb[:, s:s + 1], negparts[:], axis=AX.X)
            # thr = max_neg directly (pos blocks hold plain z from pre-pass)
            nc.vector.reduce_max(thr_sb[:, s:s + 1], maxch[:], axis=AX.X)

            msl = slice(256 * s, 256 * s + 256)
            sc1 = posp.tile([128, 256], F32, tag="sc1")
            nc.vector.scalar_tensor_tensor(
                out=sc1[:], in0=posgath[:, msl], scalar=thr_sb[:, s:s + 1],
                in1=vm_sb[:, msl], op0=OP.is_gt, op1=OP.mult,
                accum_out=corr_sb[:, s:s + 1])
            sc2 = posp.tile([128, 256], F32, tag="sc2")
            nc.vector.scalar_tensor_tensor(
                out=sc2[:], in0=posgath[:, msl], scalar=1.0,
                in1=wm_sb[:, msl], op0=OP.mult, op1=OP.mult,
                accum_out=possum_sb[:, s:s + 1])

        nc.sync.dma_start(out_d[:], out_sb[:])

    nc.compile()
    return nc


def _host_prep(feats1, feats2, overlap_inds):
    feats = np.concatenate([np.asarray(feats1, np.float32),
                            np.asarray(feats2, np.float32)], 0)
    featsT = np.ascontiguousarray(feats.T * np.float32(np.sqrt(TEMP)))
    ov = np.asarray(overlap_inds, bool)
    eye128 = np.eye(128, dtype=np.float32)

    in_maps = []
    wcnts, vcnts = [], []
    for c in range(NC):
        view2 = c >= 4
        cc = c - 4 if view2 else c
        self_s = 2048 + 512 * cc if view2 else 512 * cc
        other_s = 512 * cc if view2 else 2048 + 512 * cc
        keep = np.ones(N, bool)
        keep[self_s:self_s + 512] = False
        keep[other_s:other_s + 512] = False
        perm = np.concatenate([np.arange(self_s, self_s + 512),
                               np.arange(other_s, other_s + 512),
                               np.nonzero(keep)[0]])
        fT_c = featsT[:, perm].astype(np.float16)
        fT_c = np.ascontiguousarray(
            fT_c.reshape(F, 8, 512).transpose(1, 0, 2))

        V = np.zeros((128, NSTRIPE, 2, 128), np.float32)
        W = np.zeros((128, NSTRIPE, 2, 128), np.float32)
        ovfix = np.zeros((1, 512), np.float16)
        for s in range(NSTRIPE):
            for h in range(2):
                u = 2 * s + h
                m = 8 * cc + u
                rows = slice(64 * h, 64 * h + 64)
                lo = 64 * u - 128 * s
                V[rows, s, 0, lo:lo + 64] = 1.0
                W[rows, s, 0, lo:lo + 64] = 1.0
                if ov[m]:
                    V[rows, s, 1, lo:lo + 64] = 1.0
                    W[rows, s, 1, lo:lo + 64] = OTHER
                    ovfix[0, 64 * u:64 * u + 64] = -SQB
            V[:, s, 0, :] *= (1 - eye128)
            W[:, s, 0, :] *= (1 - eye128)

        wcnts.append(W.reshape(128, NSTRIPE, 256).sum(-1))
        vcnts.append(V.reshape(128, NSTRIPE, 256).sum(-1))
        in_maps.append({
            "featsT": fT_c,
            "vmask": np.ascontiguousarray(V.reshape(128, 1024).astype(np.float16)),
            "wmask": np.ascontiguousarray(W.reshape(128, 1024).astype(np.float16)),
            "ovfix": ovfix,
        })
    return in_maps, wcnts, vcnts


def kernel(feats1, feats2, overlap_inds, bs):
    assert int(bs) == BS
    feats1 = np.asarray(feats1, np.float32)
    feats2 = np.asarray(feats2, np.float32)
    assert feats1.shape == (N1, F) and feats2.shape == (N1, F)

    in_maps, wcnts, vcnts = _host_prep(feats1, feats2, overlap_inds)

    if "nc" not in _CACHE:
        _CACHE["nc"] = _build_nc()
    res = run_bass_kernel_spmd(_CACHE["nc"], in_maps, list(range(NC)))

    total_loss = 0.0
    total_corr = 0.0
    total_pos = 0.0
    for c in range(NC):
        out = res.results[c]["outs"]
        negsum = out[:, 0:4].astype(np.float64)
        possum = out[:, 8:12].astype(np.float64)
        corr = out[:, 12:16].astype(np.float64)
        wcnt = wcnts[c].astype(np.float64)
        total_loss += (wcnt * np.log(negsum) - possum).sum()
        total_corr += corr.sum()
        total_pos += vcnts[c].sum(dtype=np.float64)

    loss = np.float32(total_loss / total_pos)
    acc = np.float32(total_corr / total_pos)
    return acc, loss


# revision 23
# speedup vs baseline: 1.8147x; 1.0815x over previous
"""Contrastive-loss kernel for 8 Trainium2 NeuronCores (SPMD, Bass/Tile).

Strategy (data-parallel over rows of the 4096x4096 similarity matrix):
  - Each core owns 512 rows (4 stripes of 128). It receives the full feature
    matrix, column-PERMUTED per core so its positive-pair blocks sit at
    program-constant offsets: perm = [own-view 512-block, other-view
    512-block, rest]. All core-dependence lives in input data (SPMD-safe).
  - featsT is pre-scaled by sqrt(TEMP) so PE matmuls produce logits directly.
  - Per stripe: two [128,2048] PSUM groups of matmuls (float32r, full rate);
    rank-1 bf16-free fixup matmuls subtract BIG=25 on same-class blocks so
    exp/max row-reductions see negatives only.
  - ACT: exp with fused row-accumulate -> neg_sum. DVE: strided
    tensor_tensor_reduce max chain -> max_neg (as thr = max_neg - BIG).
  - Positive blocks are read straight from PSUM (post-fixup, values z-BIG):
    fused compare+count (correct) and weighted-sum (pos logits) ops.
  - Host: tiny label math, final log/sum/divide on 4096-length vectors.
"""
import sys

if "/opt/trn_rl_repo" not in sys.path:
    sys.path.insert(0, "/opt/trn_rl_repo")

from contextlib import ExitStack

import numpy as np

import concourse.bass as bass
import concourse.tile as tile
from concourse import bacc, mybir
from concourse.bass_utils import run_bass_kernel_spmd

F32 = mybir.dt.float32
AX = mybir.AxisListType
OP = mybir.AluOpType
ACTF = mybir.ActivationFunctionType

K = 32
TEMP = 0.01
OTHER = 0.5
BS = 64
F = 128
N1 = 2048
N = 4096
NC = 8
RPC = 512          # rows per core
NSTRIPE = 4
BIG = 25.0
SQB = 5.0          # sqrt(BIG)

_CACHE: dict = {}


def _build_nc():
    nc = bacc.Bacc("TRN2", target_bir_lowering=False, debug=False, num_devices=NC)

    F16 = mybir.dt.float16
    fT_d = nc.dram_tensor("featsT", [4, F, 1024], F16, kind="ExternalInput").ap()
    vm_d = nc.dram_tensor("vmask", [128, 1024], F16, kind="ExternalInput").ap()
    wm_d = nc.dram_tensor("wmask", [128, 1024], F16, kind="ExternalInput").ap()
    ovf_d = nc.dram_tensor("ovfix", [1, 512], F16, kind="ExternalInput").ap()

    out_d = nc.dram_tensor("outs", [128, 16], F32, kind="ExternalOutput").ap()

    with tile.TileContext(nc) as tc, ExitStack() as ctx:
        singles = ctx.enter_context(tc.tile_pool(name="singles", bufs=1))
        expp = ctx.enter_context(tc.tile_pool(name="expp", bufs=2))
        posp = ctx.enter_context(tc.tile_pool(name="posp", bufs=2))
        stat = ctx.enter_context(tc.tile_pool(name="stat", bufs=3))
        outp = ctx.enter_context(tc.tile_pool(name="outs", bufs=1))

        ovf_sb = singles.tile([1, 512], F16)
        nc.gpsimd.dma_start(ovf_sb[:], ovf_d[:])
        fpair = []
        pair_eng = [nc.sync, nc.scalar, nc.gpsimd, nc.gpsimd]
        for p in range(4):
            cht = singles.tile([F, 1024], F16, name=f"fpair{p}")
            pair_eng[p].dma_start(cht[:], fT_d[p])
            fpair.append(cht)
        # chunk views: fch[t] == perm cols [512t, 512t+512)
        fch = [fpair[t // 2][:, 512 * (t % 2):512 * (t % 2) + 512]
               for t in range(8)]
        vm_sb = singles.tile([128, 1024], F16)
        nc.sync.dma_start(vm_sb[:], vm_d[:])
        wm_sb = singles.tile([128, 1024], F16)
        nc.gpsimd.dma_start(wm_sb[:], wm_d[:])
        ones_pos = singles.tile([1, 64], F16)
        nc.vector.memset(ones_pos[:], SQB)
        ones_neg = singles.tile([1, 64], F16)
        nc.vector.memset(ones_neg[:], -SQB)

        out_sb = outp.tile([128, 16], F32)
        negsum_sb = out_sb[:, 0:4]
        thr_sb = out_sb[:, 4:8]
        possum_sb = out_sb[:, 8:12]
        corr_sb = out_sb[:, 12:16]

        # ---- pre-pass: positive blocks as plain-z matmuls (runs in the
        # DMA shadow; shares the main PSUM pool's slot rotation) ----
        psum = ctx.enter_context(tc.tile_pool(name="psum", bufs=4, space="PSUM"))
        posgath = singles.tile([128, 1024], F32)
        pz = psum.tile([128, 1024], F32, tag="zg", name="pz")
        for s in range(NSTRIPE):
            for b in range(2):
                nc.tensor.matmul(
                    pz[:, 256 * s + 128 * b:256 * s + 128 * b + 128],
                    fpair[0][:, 128 * s:128 * s + 128],
                    fpair[0][:, 512 * b + 128 * s:512 * b + 128 * s + 128],
                    start=True, stop=True)
        nc.scalar.copy(posgath[:], pz[:])
        for s in range(NSTRIPE):
            lhsT = fpair[0][:, 128 * s:128 * s + 128]
            zg = [psum.tile([128, 1024], F32, tag="zg", name=f"zg{s}_{g}")
                  for g in range(4)]
            # all 8 big matmuls back-to-back with the same stationary lhsT
            for g in range(4):
                for t2 in range(2):
                    nc.tensor.matmul(
                        zg[g][:, 512 * t2:512 * (t2 + 1)],
                        lhsT,
                        fpair[g][:, 512 * t2:512 * (t2 + 1)],
                        start=True, stop=True)
            # fixups: subtract BIG on same-class blocks (group 0 only);
            # emitted after all big matmuls so PE switches weights only once.
            for h in range(2):
                u = 2 * s + h
                nc.tensor.matmul(
                    zg[0][64 * h:64 * h + 64, 64 * u:64 * u + 64],
                    ones_pos[:], ones_neg[:],
                    start=False, stop=True, skip_group_check=True)
                nc.tensor.matmul(
                    zg[0][64 * h:64 * h + 64, 512 + 64 * u:512 + 64 * u + 64],
                    ones_pos[:], ovf_sb[:, 64 * u:64 * u + 64],
                    start=False, stop=True, skip_group_check=True)

            negparts = stat.tile([128, 4], F32)
            maxch = stat.tile([128, 4], F32)
            for g in range(4):
                ex = expp.tile([128, 1024], F32, tag="ex", name=f"ex{s}_{g}")
                nc.scalar.activation(ex[:], zg[g][:], ACTF.Exp,
                                     accum_out=negparts[:, g:g + 1])
                nc.vector.reduce_max(maxch[:, g:g + 1], zg[g][:], axis=AX.X)
            nc.vector.reduce_sum(negsum_sb[:, s:s + 1], negparts[:], axis=AX.X)
            # thr = max_neg directly (pos blocks hold plain z from pre-pass)
            nc.vector.reduce_max(thr_sb[:, s:s + 1], maxch[:], axis=AX.X)

            msl = slice(256 * s, 256 * s + 256)
            sc1 = posp.tile([128, 256], F32, tag="sc1")
            nc.vector.scalar_tensor_tensor(
                out=sc1[:], in0=posgath[:, msl], scalar=thr_sb[:, s:s + 1],
                in1=vm_sb[:, msl], op0=OP.is_gt, op1=OP.mult,
                accum_out=corr_sb[:, s:s + 1])
            sc2 = posp.tile([128, 256], F32, tag="sc2")
            nc.vector.scalar_tensor_tensor(
                out=sc2[:], in0=posgath[:, msl], scalar=1.0,
                in1=wm_sb[:, msl], op0=OP.mult, op1=OP.mult,
                accum_out=possum_sb[:, s:s + 1])

        nc.sync.dma_start(out_d[:], out_sb[:])

    nc.compile()
    return nc


def _host_prep(feats1, feats2, overlap_inds):
    feats = np.concatenate([np.asarray(feats1, np.float32),
                            np.asarray(feats2, np.float32)], 0)
    featsT = np.ascontiguousarray(feats.T * np.float32(np.sqrt(TEMP)))
    ov = np.asarray(overlap_inds, bool)
    eye128 = np.eye(128, dtype=np.float32)

    in_maps = []
    wcnts, vcnts = [], []
    for c in range(NC):
        view2 = c >= 4
        cc = c - 4 if view2 else c
        self_s = 2048 + 512 * cc if view2 else 512 * cc
        other_s = 512 * cc if view2 else 2048 + 512 * cc
        keep = np.ones(N, bool)
        keep[self_s:self_s + 512] = False
        keep[other_s:other_s + 512] = False
        perm = np.concatenate([np.arange(self_s, self_s + 512),
                               np.arange(other_s, other_s + 512),
                               np.nonzero(keep)[0]])
        fT_c = featsT[:, perm].astype(np.float16)
        fT_c = np.ascontiguousarray(
            fT_c.reshape(F, 4, 1024).transpose(1, 0, 2))

        V = np.zeros((128, NSTRIPE, 2, 128), np.float32)
        W = np.zeros((128, NSTRIPE, 2, 128), np.float32)
        ovfix = np.zeros((1, 512), np.float16)
        for s in range(NSTRIPE):
            for h in range(2):
                u = 2 * s + h
                m = 8 * cc + u
                rows = slice(64 * h, 64 * h + 64)
                lo = 64 * u - 128 * s
                V[rows, s, 0, lo:lo + 64] = 1.0
                W[rows, s, 0, lo:lo + 64] = 1.0
                if ov[m]:
                    V[rows, s, 1, lo:lo + 64] = 1.0
                    W[rows, s, 1, lo:lo + 64] = OTHER
                    ovfix[0, 64 * u:64 * u + 64] = -SQB
            V[:, s, 0, :] *= (1 - eye128)
            W[:, s, 0, :] *= (1 - eye128)

        wcnts.append(W.reshape(128, NSTRIPE, 256).sum(-1))
        vcnts.append(V.reshape(128, NSTRIPE, 256).sum(-1))
        in_maps.append({
            "featsT": fT_c,
            "vmask": np.ascontiguousarray(V.reshape(128, 1024).astype(np.float16)),
            "wmask": np.ascontiguousarray(W.reshape(128, 1024).astype(np.float16)),
            "ovfix": ovfix,
        })
    return in_maps, wcnts, vcnts


def kernel(feats1, feats2, overlap_inds, bs):
    assert int(bs) == BS
    feats1 = np.asarray(feats1, np.float32)
    feats2 = np.asarray(feats2, np.float32)
    assert feats1.shape == (N1, F) and feats2.shape == (N1, F)

    in_maps, wcnts, vcnts = _host_prep(feats1, feats2, overlap_inds)

    if "nc" not in _CACHE:
        _CACHE["nc"] = _build_nc()
    res = run_bass_kernel_spmd(_CACHE["nc"], in_maps, list(range(NC)))

    total_loss = 0.0
    total_corr = 0.0
    total_pos = 0.0
    for c in range(NC):
        out = res.results[c]["outs"]
        negsum = out[:, 0:4].astype(np.float64)
        possum = out[:, 8:12].astype(np.float64)
        corr = out[:, 12:16].astype(np.float64)
        wcnt = wcnts[c].astype(np.float64)
        total_loss += (wcnt * np.log(negsum) - possum).sum()
        total_corr += corr.sum()
        total_pos += vcnts[c].sum(dtype=np.float64)

    loss = np.float32(total_loss / total_pos)
    acc = np.float32(total_corr / total_pos)
    return acc, loss
